# revision 1
# baseline (speedup 1.0000x reference)
"""LDA-loss logits kernel for Trainium2 (8 NeuronCores, SPMD).

Computes logits[b, c] = -0.5 * ||feat[b] - centers[c]||^2
                      = feat[b]·centers[c] - 0.5||feat[b]||^2 - 0.5||centers[c]||^2

Strategy:
  - Shard centers (output columns) across the 8 cores: 10000 classes ->
    1250/core (matmul N-tiles of 512+512+226).
  - Host prep: transpose feat/centers to [D, *] bf16 layouts (contraction on
    partitions), precompute the two squared-norm bias vectors in fp32.
  - Device: everything SBUF-resident. 8x128 K-chunks of bf16 matmuls
    accumulate in fp32 PSUM; eviction adds the per-row bias on ScalarE
    (activation Identity + per-partition bias) and the per-column bias on
    VectorE, then DMAs the fp32 tile out.
"""

import numpy as np
import ml_dtypes

BATCH = 4096
FEAT_DIM = 1024
NUM_CLASSES = 10000
N_CORES = 8
C_PER_REAL = NUM_CLASSES // N_CORES  # 1250
C_PER = 1250                         # padded per-core classes
P = 128
KO = FEAT_DIM // P                   # 8 contraction chunks
MT = BATCH // P                      # 32 output row tiles
N_TILES = ((0, 512), (512, 512), (1024, 226))

_NC = None


def _build_bass():
    import concourse.mybir as mybir
    import concourse.tile as tile
    from concourse import bacc

    nc = bacc.Bacc("TRN2", target_bir_lowering=False, debug=False)

    featT = nc.dram_tensor("featT", [FEAT_DIM, BATCH], mybir.dt.bfloat16,
                           kind="ExternalInput")
    centsT = nc.dram_tensor("centsT", [FEAT_DIM, C_PER], mybir.dt.bfloat16,
                            kind="ExternalInput")
    fsq = nc.dram_tensor("fsq", [P, MT], mybir.dt.float32, kind="ExternalInput")
    csqb = nc.dram_tensor("csqb", [P, C_PER], mybir.dt.float32,
                          kind="ExternalInput")
    out = nc.dram_tensor("out", [BATCH, C_PER], mybir.dt.float32,
                         kind="ExternalOutput")

    with tile.TileContext(nc) as tc:
        _lda_tile_kernel(tc, featT.ap(), centsT.ap(), fsq.ap(), csqb.ap(),
                         out.ap())
    nc.compile()
    return nc


def _lda_tile_kernel(tc, featT, centsT, fsq, csqb, out):
    import concourse.mybir as mybir

    nc = tc.nc
    featT_r = featT.rearrange("(ko p) b -> p ko b", p=P)
    centsT_r = centsT.rearrange("(ko p) c -> p ko c", p=P)
    out_r = out.rearrange("(mo p) c -> p mo c", p=P)

    with (
        tc.tile_pool(name="big", bufs=1) as big,
        tc.tile_pool(name="consts", bufs=1) as consts,
        tc.tile_pool(name="ostage", bufs=16) as ostage,
        tc.tile_pool(name="psum", bufs=6, space="PSUM") as psum,
    ):
        cent_sb = big.tile([P, KO, C_PER], mybir.dt.bfloat16)
        feat_sb = big.tile([P, KO, BATCH], mybir.dt.bfloat16)
        fsq_sb = consts.tile([P, MT], mybir.dt.float32)
        csq_sb = consts.tile([P, C_PER], mybir.dt.float32)

        # All input loads go on the HW-DGE (sync) queue, which sustains full
        # HBM bandwidth when it runs alone. Centers + the first feat m-range
        # load in REVERSE k order: the first matmul needs k=0, which arrives
        # last, so TensorE starts only once its whole first working set is
        # resident and then runs dense — drip-feeding it causes HAM
        # re-throttle stalls that cost more than the later start saves.
        # Output stores use the SW-DGE (gpsimd) queue so they never wait
        # behind the input load in one FIFO.
        MR = 8
        mr_size = BATCH // MR
        for k in range(KO - 1, -1, -1):
            nc.sync.dma_start(cent_sb[:, k], centsT_r[:, k])
            nc.gpsimd.dma_start(feat_sb[:, k, 0:mr_size],
                                featT_r[:, k, 0:mr_size])
        nc.sync.dma_start(fsq_sb[:], fsq)
        nc.sync.dma_start(csq_sb[:], csqb)
        for mr in range(1, MR):
            sl = slice(mr * mr_size, (mr + 1) * mr_size)
            for k in range(KO):
                nc.sync.dma_start(feat_sb[:, k, sl], featT_r[:, k, sl])

        for m in range(MT):
            msl = slice(m * P, (m + 1) * P)
            for n0, nsz in N_TILES:
                ps = psum.tile([P, 512], mybir.dt.float32, tag="ps",
                               name="ps")[:, :nsz]
                for k in range(KO):
                    nc.tensor.matmul(
                        ps,
                        feat_sb[:, k, msl],
                        cent_sb[:, k, n0:n0 + nsz],
                        start=(k == 0),
                        stop=(k == KO - 1),
                    )
                ot = ostage.tile([P, 512], mybir.dt.float32, tag="ot",
                                 name="ot")[:, :nsz]
                # ot = psum + fsq[row]  (per-partition bias on ScalarE)
                nc.scalar.activation(
                    ot, ps, mybir.ActivationFunctionType.Identity,
                    bias=fsq_sb[:, m:m + 1],
                )
                # ot += csq[col]  (per-column bias on VectorE)
                nc.vector.tensor_add(ot, ot, csq_sb[:, n0:n0 + nsz])
                eng = nc.gpsimd if (m + n0) % 2 else nc.sync
                eng.dma_start(out_r[:, m, n0:n0 + nsz], ot)


def _get_nc():
    global _NC
    if _NC is None:
        _NC = _build_bass()
    return _NC


def _prep_inputs(feat, centers):
    feat = np.asarray(feat, dtype=np.float32)
    centers = np.asarray(centers, dtype=np.float32)

    featT_bf = np.ascontiguousarray(feat.T).astype(ml_dtypes.bfloat16)
    fsq_v = -0.5 * np.einsum("bd,bd->b", feat, feat)
    fsq_mat = np.ascontiguousarray(fsq_v.reshape(MT, P).T)  # [P, MT]

    in_maps = []
    for i in range(N_CORES):
        cs = centers[i * C_PER_REAL:(i + 1) * C_PER_REAL]
        centsT_bf = np.zeros((FEAT_DIM, C_PER), dtype=ml_dtypes.bfloat16)
        centsT_bf[:, :C_PER_REAL] = cs.T.astype(ml_dtypes.bfloat16)
        csq = np.zeros(C_PER, dtype=np.float32)
        csq[:C_PER_REAL] = -0.5 * np.einsum("cd,cd->c", cs, cs)
        csqb = np.ascontiguousarray(
            np.broadcast_to(csq[None, :], (P, C_PER)))
        in_maps.append({
            "featT": featT_bf,
            "centsT": centsT_bf,
            "fsq": fsq_mat,
            "csqb": csqb,
        })
    return in_maps


def _run(inputs, trace=False, trace_cores=None):
    from concourse import bass_utils

    nc = _get_nc()
    in_maps = _prep_inputs(inputs["feat"], inputs["centers"])
    res = bass_utils.run_bass_kernel_spmd(
        nc, in_maps, core_ids=list(range(N_CORES)), trace=trace,
        trace_cores=trace_cores,
    )
    shards = [res.results[i]["out"][:, :C_PER_REAL] for i in range(N_CORES)]
    full = np.concatenate(shards, axis=1)
    return full, res


def kernel(**inputs) -> np.ndarray:
    return _run(inputs)[0]



# revision 2
# speedup vs baseline: 1.6862x; 1.6862x over previous
"""LDA-loss logits kernel for Trainium2 (8 NeuronCores, SPMD).

Computes logits[b, c] = -0.5 * ||feat[b] - centers[c]||^2
                      = feat[b]·centers[c] - 0.5||feat[b]||^2 - 0.5||centers[c]||^2

Strategy (v2, fp8 DoubleRow):
  - Shard feat over batch: 4096 rows -> 512/core (4 m-tiles of 128), centers
    replicated.  Classes padded 10000 -> 10240 = 20 n-tiles of 512 so every
    matmul streams a full 512-wide moving operand (hides LDWEIGHTS).
  - Inputs quantized to fp8e4 on host; matmuls run perf_mode=DoubleRow
    (2 contraction chunks of 128 per pass -> ~1.8x bf16 throughput).  The two
    squared-norm biases are fp32/fp16 host-precomputed; the fp8 error only
    touches the cross term (~7e-3 rel, tolerance 2e-2).
  - n-outer loop: center columns stream in per n-tile so the first matmul
    issues after ~1MB of DMA instead of the whole working set.
  - Eviction: ScalarE adds the per-row bias (fp32 PSUM -> fp16 SBUF), VectorE
    adds the per-column bias (fp16, 2x DVE rate), output DMAs as fp16 and the
    host upcasts to fp32 after the gather (halves output HBM traffic).
"""

import numpy as np
import ml_dtypes

BATCH = 4096
FEAT_DIM = 1024
NUM_CLASSES = 10000
N_CORES = 8
B_PER = BATCH // N_CORES            # 512 rows per core
P = 128
MT = B_PER // P                     # 4 output row tiles per core
KO = FEAT_DIM // P                  # 8 contraction chunks
KP = KO // 2                        # 4 DoubleRow chunk-pairs
NT = 20                             # n-tiles of 512
C_PAD = NT * 512                    # 10240 padded classes

_NC = None


def _build_bass():
    import concourse.mybir as mybir
    import concourse.tile as tile
    from concourse import bacc

    nc = bacc.Bacc("TRN2", target_bir_lowering=False, debug=False)

    featT = nc.dram_tensor("featT", [P, KO * B_PER], mybir.dt.float8e4,
                           kind="ExternalInput")
    centsT = nc.dram_tensor("centsT", [NT, P, KO * 512], mybir.dt.float8e4,
                            kind="ExternalInput")
    fsq = nc.dram_tensor("fsq", [P, MT], mybir.dt.float32, kind="ExternalInput")
    csq = nc.dram_tensor("csq", [NT, P, 512], mybir.dt.float16,
                         kind="ExternalInput")
    out = nc.dram_tensor("out", [B_PER, C_PAD], mybir.dt.float16,
                         kind="ExternalOutput")

    with tile.TileContext(nc) as tc:
        _lda_tile_kernel(tc, featT.ap(), centsT.ap(), fsq.ap(), csq.ap(),
                         out.ap())
    nc.compile()
    return nc


def _lda_tile_kernel(tc, featT, centsT, fsq, csq, out):
    import concourse.mybir as mybir

    nc = tc.nc
    out_r = out.rearrange("(mo p) c -> p mo c", p=P)

    with (
        tc.tile_pool(name="big", bufs=1) as big,
        tc.tile_pool(name="consts", bufs=1) as consts,
        tc.tile_pool(name="ostage", bufs=8) as ostage,
        tc.tile_pool(name="psum", bufs=8, space="PSUM") as psum,
    ):
        cent_sb = big.tile([P, NT, KO, 512], mybir.dt.float8e4)
        feat_sb = big.tile([P, KO, B_PER], mybir.dt.float8e4)
        csq_sb = consts.tile([P, NT, 512], mybir.dt.float16)
        fsq_sb = consts.tile([P, MT], mybir.dt.float32)

        # Input loads on the HW-DGE (sync) queue in consumption order: the
        # first matmul needs only feat + the first center n-tile (~1MB), so
        # the tensor engine starts ~3us in instead of waiting for the full
        # working set.  Output stores go on the SW-DGE (gpsimd) queue so they
        # never queue behind input loads.
        nc.sync.dma_start(fsq_sb[:], fsq)
        nc.sync.dma_start(feat_sb[:],
                          featT.rearrange("p (ko b) -> p ko b", ko=KO))
        for j in range(NT):
            nc.sync.dma_start(cent_sb[:, j],
                              centsT[j].rearrange("p (ko c) -> p ko c", ko=KO))
            nc.sync.dma_start(csq_sb[:, j], csq[j])

        for j in range(NT):
            ps = [psum.tile([P, 512], mybir.dt.float32, tag="ps", name="ps")
                  for _ in range(MT)]
            for kp in range(KP):
                for m in range(MT):
                    nc.tensor.matmul(
                        ps[m],
                        feat_sb[:, 2 * kp:2 * kp + 2, m * P:(m + 1) * P],
                        cent_sb[:, j, 2 * kp:2 * kp + 2, :],
                        start=(kp == 0),
                        stop=(kp == KP - 1),
                        perf_mode=mybir.MatmulPerfMode.DoubleRow,
                    )
            for m in range(MT):
                ot = ostage.tile([P, 512], mybir.dt.float16, tag="ot",
                                 name="ot")
                # ot = psum + fsq[row]  (per-partition bias on ScalarE)
                nc.scalar.activation(
                    ot, ps[m], mybir.ActivationFunctionType.Identity,
                    bias=fsq_sb[:, m:m + 1],
                )
                # ot += csq[col]  (per-column bias on VectorE, fp16 2x rate)
                nc.vector.tensor_add(ot, ot, csq_sb[:, j])
                nc.gpsimd.dma_start(out_r[:, m, j * 512:(j + 1) * 512], ot)


def _get_nc():
    global _NC
    if _NC is None:
        _NC = _build_bass()
    return _NC


def _prep_inputs(feat, centers):
    feat = np.asarray(feat, dtype=np.float32)
    centers = np.asarray(centers, dtype=np.float32)
    f8 = ml_dtypes.float8_e4m3

    cent_pad = np.zeros((C_PAD, FEAT_DIM), dtype=np.float32)
    cent_pad[:NUM_CLASSES] = centers
    # centsT_sw[j, p, ko*512 + c] = centers[j*512 + c, ko*128 + p]
    centsT_sw = np.ascontiguousarray(
        cent_pad.T.astype(f8).reshape(KO, P, NT, 512).transpose(2, 1, 0, 3)
    ).reshape(NT, P, KO * 512)

    csq_v = np.zeros(C_PAD, dtype=np.float32)
    csq_v[:NUM_CLASSES] = -0.5 * np.einsum("cd,cd->c", centers, centers)
    csq_sw = np.ascontiguousarray(np.broadcast_to(
        csq_v.astype(np.float16).reshape(NT, 1, 512), (NT, P, 512)))

    feat8 = feat.astype(f8)
    fsq_v = -0.5 * np.einsum("bd,bd->b", feat, feat)

    in_maps = []
    for i in range(N_CORES):
        r0 = i * B_PER
        # featT_sw[p, ko*512 + m] = feat[r0 + m, ko*128 + p]
        featT_sw = np.ascontiguousarray(
            feat8[r0:r0 + B_PER].T.reshape(KO, P, B_PER).transpose(1, 0, 2)
        ).reshape(P, KO * B_PER)
        fsq_mat = np.ascontiguousarray(
            fsq_v[r0:r0 + B_PER].reshape(MT, P).T)
        in_maps.append({
            "featT": featT_sw,
            "centsT": centsT_sw,
            "fsq": fsq_mat,
            "csq": csq_sw,
        })
    return in_maps


def _run(inputs, trace=False, trace_cores=None):
    from concourse import bass_utils

    nc = _get_nc()
    in_maps = _prep_inputs(inputs["feat"], inputs["centers"])
    res = bass_utils.run_bass_kernel_spmd(
        nc, in_maps, core_ids=list(range(N_CORES)), trace=trace,
        trace_cores=trace_cores,
    )
    full = np.concatenate(
        [np.asarray(res.results[i]["out"]) for i in range(N_CORES)], axis=0)
    return full[:, :NUM_CLASSES].astype(np.float32), res


def kernel(**inputs) -> np.ndarray:
    return _run(inputs)[0]
